# revision 1
# baseline (speedup 1.0000x reference)
"""Trainium2 Bass kernel for nn_CompressorModel (block decompression + linear head).

The reference computes, per sample b:
    y[b] = W . vec(stack_ch(lhs[r] @ block[r,c] @ rhs[c])) + bias
which is linear in x.  We fold (lhs, rhs, W) into a single effective weight
vector W_eff[768] on the host (fp64), reducing the device work to a pure
memory-bound matvec:  y = x.reshape(B, 768) @ W_eff + bias.

Device strategy (pure data parallel, batch sharded 8 ways). Per core the
shard [4096, 768] is viewed as [512, 6144] (partition line = 8 consecutive
rows, 24KB contiguous DRAM) and split into 32 "units" (one unit = 768 elems
on 128 partitions = 512 rows' worth of one row-position = 384KB).

Pipeline (raw bass; this walrus build rejects >1 sync-wait per instruction,
so every cross-engine dep is a standalone wait_ge on its own semaphore):
  SP     : HWDGE x-tile DMAs on a tapered schedule (1-2 units per DMA,
           single units at the edges to shrink startup/tail), plus the
           final output DMA.
  ScalarE ring: w [128,768] + bias-lane loads (overlaps SP's first gens).
  VectorE: one tensor_mul per unit into a 769-lane-strided product ring;
           lane 768 holds the bias (preloaded once), so each row dot
           product + bias is exactly a 769-lane sum. The final unit is
           reduced on VectorE itself to shorten the tail.
  ScalarE: one activation(Copy, accum_out) per unit over 769 lanes ->
           res[:, unit] = sum = y row value (bias included). No adds, no
           self-waits on the critical path.
  GPSIMD : output DMAs [128, 8] per partition-block, off the compute path.
"""

import numpy as np

B = 32768
N_CORES = 8
B_PER = B // N_CORES          # 4096 rows per core
F = 768                       # 3*16*16 features per row
L = F + 1                     # product lanes per unit (768 + bias lane)
RPP = 8                       # rows packed per partition line
NPL = B_PER // RPP            # 512 partition lines per core
LINE = RPP * F                # 6144 elems per partition line
P = 128                       # SBUF partitions
UNITS = 32                    # units per core (one unit = [128, 768])
RING = 16                     # ring capacity in units (x and product rings)
# Tapered tile schedule in units; sums to 32, no tile crosses an 8-unit
# partition block, ring placement (g % 16) never wraps.
TILES = [1, 1, 2, 2, 2, 2, 2, 2, 2, 2, 2, 2, 2, 2, 2, 1, 1, 1, 1]
NXSEM = 8                     # rotating x-DMA completion sems
NOSEM = 2                     # rotating out-DMA completion sems

_cache = {}


def _fold_weights(lhs, rhs, W):
    """W_eff[ch, r*8+p, c*8+q] = sum_{P,Q} lhs[r,P,p]*rhs[c,q,Q]*W[0, ch*1024+(r*16+P)*32+(c*16+Q)]"""
    Wb = np.asarray(W, np.float64).reshape(3, 2, 16, 2, 16)  # [ch, r, P, c, Q]
    weff = np.einsum(
        "rPp,cqQ,nrPcQ->nrpcq",
        np.asarray(lhs, np.float64),
        np.asarray(rhs, np.float64),
        Wb,
    )
    return np.ascontiguousarray(weff.reshape(F)).astype(np.float32)


def _build_program(reps=1):
    """Build the per-core program. reps>1 streams the same shard `reps` times
    (identical output, used only for wall-clock timing harnesses)."""
    key = ("nc", reps)
    if key in _cache:
        return _cache[key]
    from concourse import bass, mybir

    f32 = mybir.dt.float32
    nc = bass.Bass("TRN2", target_bir_lowering=False, debug=False)
    xs = nc.dram_tensor("xs", [NPL, LINE], f32, kind="ExternalInput").ap()
    wf = nc.dram_tensor("wf", [P, F], f32, kind="ExternalInput").ap()
    bs = nc.dram_tensor("bs", [P, RING], f32, kind="ExternalInput").ap()
    ys = nc.dram_tensor("ys", [NPL, RPP], f32, kind="ExternalOutput").ap()

    w_t = nc.alloc_sbuf_tensor("w_t", [P, F], f32).ap()
    xb = nc.alloc_sbuf_tensor("xb", [P, RING * F], f32).ap()
    pb = nc.alloc_sbuf_tensor("pb", [P, RING * L], f32).ap()
    res = nc.alloc_sbuf_tensor("res", [P, UNITS], f32).ap()

    pbv = pb.rearrange("p (s l) -> p s l", s=RING)

    # --- static schedule tables (global over reps) ---
    tiles = []  # (t, g_global, k)
    g = 0
    for r in range(reps):
        for k in TILES:
            tiles.append((len(tiles), g, k))
            g += k
    n_blocks = 4 * reps
    T_UNITS = UNITS * reps

    def xs_src(gg, k):
        u = gg % UNITS
        bl, c = divmod(u, RPP)
        return xs[bl * P : (bl + 1) * P, c * F : (c + k) * F]

    def xb_slot(gg, k):
        rs = gg % RING
        return xb[:, rs * F : (rs + k) * F]

    with (
        nc.Block() as block,
        nc.semaphore("s_w") as s_w,
        nc.semaphore("s_b") as s_b,
        nc.semaphore("s_x0") as s_x0,
        nc.semaphore("s_x1") as s_x1,
        nc.semaphore("s_x2") as s_x2,
        nc.semaphore("s_x3") as s_x3,
        nc.semaphore("s_x4") as s_x4,
        nc.semaphore("s_x5") as s_x5,
        nc.semaphore("s_x6") as s_x6,
        nc.semaphore("s_x7") as s_x7,
        nc.semaphore("s_v") as s_v,
        nc.semaphore("s_a") as s_a,
        nc.semaphore("s_hb") as s_hb,
        nc.semaphore("s_of") as s_of,
        nc.semaphore("s_o0") as s_o0,
        nc.semaphore("s_o1") as s_o1,
    ):
        s_x = [s_x0, s_x1, s_x2, s_x3, s_x4, s_x5, s_x6, s_x7]
        s_o = [s_o0, s_o1]

        @block.sync
        def _(sync: bass.BassEngine):
            sync.dma_start(out=w_t, in_=wf).then_inc(s_w, 16)
            for t, gg, k in tiles:
                if t >= NXSEM:
                    # updater order: previous DMA on this sem lane completed
                    sync.wait_ge(s_x[t % NXSEM], 16 * (t // NXSEM))
                if gg >= RING:
                    # DVE consumed the units previously occupying this range
                    sync.wait_ge(s_v, gg + k - RING)
                if t == len(tiles) - 1:
                    # final unit split in half: first half's mult overlaps the
                    # second half's transfer
                    src = xs_src(gg, k)
                    dst = xb_slot(gg, k)
                    sync.dma_start(
                        out=dst[:, 0 : F // 2], in_=src[:, 0 : F // 2]
                    ).then_inc(s_x[t % NXSEM], 16)
                    sync.dma_start(
                        out=dst[:, F // 2 : F], in_=src[:, F // 2 : F]
                    ).then_inc(s_hb, 16)
                else:
                    sync.dma_start(out=xb_slot(gg, k), in_=xs_src(gg, k)).then_inc(
                        s_x[t % NXSEM], 16
                    )
            bl = n_blocks - 1
            sync.wait_ge(s_a, RPP * (bl + 1))
            if bl >= NOSEM:
                sync.wait_ge(s_o[bl % NOSEM], 16 * (bl // NOSEM))
            h = (bl % 4) * RPP
            sync.dma_start(
                out=ys[(bl % 4) * P : (bl % 4 + 1) * P, :],
                in_=res[:, h : h + RPP],
            ).then_inc(s_of, 16)
            n_pool = n_blocks - 1
            sync.wait_ge(s_of, 16)
            sync.wait_ge(s_o0, 16 * ((n_pool + 1) // 2))
            sync.wait_ge(s_o1, 16 * (n_pool // 2))

        @block.vector
        def _(vec: bass.BassEngine):
            vec.wait_ge(s_w, 16)
            vec.wait_ge(s_b, 16)
            for t, gg, k in tiles:
                vec.wait_ge(s_x[t % NXSEM], 16 * (t // NXSEM + 1))
                for u in range(gg, gg + k):
                    sl = u % RING
                    if u >= RING:
                        # ACT consumed the product unit previously in this slot
                        vec.wait_ge(s_a, u - RING + 1)
                    if u == T_UNITS - 1:
                        # final unit: two half mults (second waits its own
                        # half-DMA), then reduce on DVE, skipping the ACT hop
                        vec.tensor_mul(
                            pb[:, sl * L : sl * L + F // 2],
                            xb[:, sl * F : sl * F + F // 2],
                            w_t[:, 0 : F // 2],
                        ).then_inc(s_v, 1)
                        vec.wait_ge(s_hb, 16)
                        vec.tensor_mul(
                            pb[:, sl * L + F // 2 : sl * L + F],
                            xb[:, sl * F + F // 2 : (sl + 1) * F],
                            w_t[:, F // 2 : F],
                        ).then_inc(s_v, 1)
                        vec.wait_ge(s_v, T_UNITS + 1)
                        vec.tensor_reduce(
                            res[:, UNITS - 1 : UNITS],
                            pb[:, sl * L : (sl + 1) * L],
                            axis=mybir.AxisListType.X,
                            op=mybir.AluOpType.add,
                        ).then_inc(s_a, 1)
                    else:
                        vec.tensor_mul(
                            pb[:, sl * L : sl * L + F],
                            xb[:, sl * F : (sl + 1) * F],
                            w_t,
                        ).then_inc(s_v, 1)

        @block.scalar
        def _(act: bass.BassEngine):
            from concourse import mybir as mb

            with nc.allow_non_contiguous_dma(reason="tiny one-time bias lanes"):
                act.dma_start(out=pbv[:, :, F : F + 1], in_=bs).then_inc(s_b, 16)
            act.wait_ge(s_b, 16)
            for u in range(T_UNITS - 1):
                sl = u % RING
                rc = u % UNITS
                act.wait_ge(s_v, u + 1)
                if u >= RING:
                    # our own in-place writes to this slot were published via
                    # s_a; lagging wait (never stalls in steady state)
                    act.wait_ge(s_a, u - RING + 1)
                if u % RPP == 0 and u // RPP >= 4:
                    # res block reused (reps>1): its output DMA must be done
                    bl = u // RPP
                    act.wait_ge(s_o[bl % NOSEM], 16 * ((bl - 4) // NOSEM + 1))
                act.activation(
                    pb[:, sl * L : (sl + 1) * L],
                    pb[:, sl * L : (sl + 1) * L],
                    mb.ActivationFunctionType.Copy,
                    accum_out=res[:, rc : rc + 1],
                ).then_inc(s_a, 1)

        @block.gpsimd
        def _(gp: bass.BassEngine):
            for bl in range(n_blocks - 1):
                gp.wait_ge(s_a, RPP * (bl + 1))
                if bl >= NOSEM:
                    gp.wait_ge(s_o[bl % NOSEM], 16 * (bl // NOSEM))
                h = (bl % 4) * RPP
                gp.dma_start(
                    out=ys[(bl % 4) * P : (bl % 4 + 1) * P, :],
                    in_=res[:, h : h + RPP],
                ).then_inc(s_o[bl % NOSEM], 16)

    _cache[key] = nc
    return nc


def _make_in_maps(x, lhs, rhs, W, b):
    weff = _fold_weights(lhs, rhs, W)
    wf = np.ascontiguousarray(np.broadcast_to(weff, (P, F)))
    bval = np.float32(np.asarray(b, np.float32).reshape(-1)[0])
    bs = np.full((P, RING), bval, np.float32)
    xr = np.ascontiguousarray(np.asarray(x, np.float32).reshape(B, F))
    in_maps = []
    for c in range(N_CORES):
        shard = xr[c * B_PER : (c + 1) * B_PER].reshape(NPL, LINE)
        in_maps.append({"xs": shard, "wf": wf, "bs": bs})
    return in_maps


def _run(x, lhs, rhs, W, b, reps=1, **kwargs):
    from concourse.bass_utils import run_bass_kernel_spmd

    nc = _build_program(reps)
    in_maps = _make_in_maps(x, lhs, rhs, W, b)
    br = run_bass_kernel_spmd(nc, in_maps, list(range(N_CORES)), **kwargs)
    y = np.concatenate([r["ys"].reshape(B_PER) for r in br.results])
    return y.reshape(B, 1).astype(np.float32), br


def kernel(x, lhs, rhs, W, b):
    try:
        y, _ = _run(x, lhs, rhs, W, b)
    except Exception:
        # transient NRT/axon failures have been observed to clear on retry
        y, _ = _run(x, lhs, rhs, W, b)
    return y



# revision 3
# speedup vs baseline: 1.0688x; 1.0688x over previous
"""Trainium2 Bass kernel for nn_CompressorModel (block decompression + linear head).

The reference is linear in x: y[b] = W_eff . x_row[b] + bias, with W_eff[768]
folded on the host from (lhs, rhs, W) in fp64.  Device work is the pure
memory-bound matvec res = X @ W_eff, batch-sharded 8 ways (4096 rows/core).

Per-core program (raw bass, one sync-wait per instruction):
  SP    : HWDGE x-tile DMAs, tapered 2-unit -> 1-unit -> half -> quarter so the
          DVE lag drains before the tail; plus the early output DMA (units
          0..23) once s_res reaches 24.
  Pool  : w-vector DMA [1,896] (w_eff 768 + ones 128, ~10ns of bus), kvwb
          PREPARE for the final output block, and the end-of-stream
          trigger_dma -- the triggered path skips the 565+625+650ns DMA issue
          pipeline on the critical tail.
  PE    : broadcasts w to 128 partitions via ones^T @ w into PSUM (replaces a
          1092ns [128,768] broadcast DMA with a 10ns [1,896] load).
  ACT   : copies PSUM -> w_t[128,768].
  DVE   : one fused tensor_tensor_reduce per chunk: product to scratch,
          accum_out=res[:,u] with scalar-chaining for sub-unit chunks
          (scalar=res[:,u] carries the partial).  861ns/unit vs 1092ns/unit
          arrival, so DVE trails the DMA stream and the tail is just
          900 (sem prop) + one 192-lane TTR + trigger + fire.

A unit u = (block bl=u//8, row-pos c=u%8): 128 partitions = lines
bl*128..bl*128+127 of xs[512, 6144], free = line cols [c*768,(c+1)*768).
res[p, u] = dot of batch row (bl*128+p)*8 + c.  Bias is added on the host.
"""

from contextlib import ExitStack

import numpy as np

B = 32768
N_CORES = 8
B_PER = B // N_CORES          # 4096 rows per core
F = 768                       # 3*16*16 features per row
RPP = 8                       # rows packed per partition line
NPL = B_PER // RPP            # 512 partition lines per core
LINE = RPP * F                # 6144 elems per partition line
P = 128                       # SBUF partitions
UNITS = 32
NXSEM = 8                     # rotating x-DMA completion sems

# x-DMA schedule: (unit, lane_start, lane_count) per DMA, bus order.
# 2-unit tiles are encoded as (unit, 0, 1536).  All chunks keep the DRAM
# contiguous run >= 512B so the cost model charges flat 360GB/s.
X_CHUNKS = []
for j in range(8):                       # units 0..15: 2-unit tiles
    X_CHUNKS.append((2 * j, 0, 2 * F))
for u in range(16, 24):                  # units 16..23: 1-unit tiles
    X_CHUNKS.append((u, 0, F))
for u in range(24, 30):                  # units 24..29: halves
    X_CHUNKS.append((u, 0, F // 2))
    X_CHUNKS.append((u, F // 2, F // 2))
for u in range(30, 32):                  # units 30..31: quarters
    for q in range(4):
        X_CHUNKS.append((u, q * (F // 4), F // 4))
N_XDMA = len(X_CHUNKS)

_cache = {}


def _fold_weights(lhs, rhs, W):
    """W_eff[ch, r*8+p, c*8+q] = sum_{P,Q} lhs[r,P,p]*rhs[c,q,Q]*W[0, ch*1024+(r*16+P)*32+(c*16+Q)]"""
    Wb = np.asarray(W, np.float64).reshape(3, 2, 16, 2, 16)  # [ch, r, P, c, Q]
    weff = np.einsum(
        "rPp,cqQ,nrPcQ->nrpcq",
        np.asarray(lhs, np.float64),
        np.asarray(rhs, np.float64),
        Wb,
    )
    return np.ascontiguousarray(weff.reshape(F)).astype(np.float32)


def _build_program():
    if "nc" in _cache:
        return _cache["nc"]
    from concourse import bass, mybir

    f32 = mybir.dt.float32
    i32 = mybir.dt.int32
    ALU = mybir.AluOpType

    nc = bass.Bass("TRN2", target_bir_lowering=False, debug=False)
    xs = nc.dram_tensor("xs", [NPL, LINE], f32, kind="ExternalInput").ap()
    wf = nc.dram_tensor("wf", [1, F + P], f32, kind="ExternalInput").ap()
    ys1 = nc.dram_tensor("ys1", [P, 24], f32, kind="ExternalOutput").ap()
    ys2 = nc.dram_tensor("ys2", [P, 8], f32, kind="ExternalOutput").ap()

    xb = nc.alloc_sbuf_tensor("xb", [P, UNITS * F], f32).ap()
    w_sb = nc.alloc_sbuf_tensor("w_sb", [1, F + P], f32).ap()
    w_t = nc.alloc_sbuf_tensor("w_t", [P, F], f32).ap()
    res = nc.alloc_sbuf_tensor("res", [P, UNITS], f32).ap()
    scr = nc.alloc_sbuf_tensor("scr", [P, 2 * F], f32).ap()
    idx = nc.alloc_sbuf_tensor("idx", [P, 1], i32).ap()

    p1 = nc.alloc_psum_tensor("p1", [P, 512], f32).ap()
    p2 = nc.alloc_psum_tensor("p2", [P, F - 512], f32).ap()

    # 4D views for kv_writeback: out [batch=1, dhi=128, dho=8, nctx=1],
    # in [dhi=128, dho=8, batch=1, ncn=1].
    ys2_4d = ys2.rearrange("(a p) (f b) -> a p f b", a=1, b=1)
    res_4d = res[:, 24:32].rearrange("p (f a b) -> p f a b", a=1, b=1)

    def xs_src(u, ls, n):
        bl, c = divmod(u, RPP)
        return xs[bl * P : (bl + 1) * P, c * F + ls : c * F + ls + n]

    with (
        nc.Block() as block,
        nc.semaphore("s_x0") as s_x0,
        nc.semaphore("s_x1") as s_x1,
        nc.semaphore("s_x2") as s_x2,
        nc.semaphore("s_x3") as s_x3,
        nc.semaphore("s_x4") as s_x4,
        nc.semaphore("s_x5") as s_x5,
        nc.semaphore("s_x6") as s_x6,
        nc.semaphore("s_x7") as s_x7,
        nc.semaphore("s_w") as s_w,
        nc.semaphore("s_mm") as s_mm,
        nc.semaphore("s_wt") as s_wt,
        nc.semaphore("s_res") as s_res,
        nc.semaphore("s_prep") as s_prep,
        nc.semaphore("s_kv") as s_kv,
        nc.semaphore("s_o1") as s_o1,
    ):
        s_x = [s_x0, s_x1, s_x2, s_x3, s_x4, s_x5, s_x6, s_x7]

        @block.sync
        def _(sp: bass.BassEngine):
            for t, (u, ls, n) in enumerate(X_CHUNKS):
                if t >= NXSEM:
                    # updater order on the rotating sem lane
                    sp.wait_ge(s_x[t % NXSEM], 16 * (t // NXSEM))
                sp.dma_start(
                    out=xb[:, u * F + ls : u * F + ls + n], in_=xs_src(u, ls, n)
                ).then_inc(s_x[t % NXSEM], 16)
            sp.wait_ge(s_res, 24)
            sp.dma_start(out=ys1, in_=res[:, 0:24]).then_inc(s_o1, 16)
            sp.wait_ge(s_o1, 16)

        @block.gpsimd
        def _(gp: bass.BassEngine):
            gp.dma_start(out=w_sb, in_=wf).then_inc(s_w, 16)
            gp.memset(idx, 0)
            gp.kv_writeback(
                ys2_4d, res_4d, idx, prepare_only=True, sem=s_kv
            ).then_inc(s_prep, 1)
            gp.wait_ge(s_prep, 1)
            gp.wait_ge(s_res, len(X_CHUNKS) + 8)  # all TTRs done (see DVE count)
            gp.trigger_dma(count=1)
            gp.wait_ge(s_kv, 16)

        @block.tensor
        def _(pe: bass.BassEngine):
            pe.wait_ge(s_w, 16)
            pe.matmul(p1, w_sb[0:1, F : F + P], w_sb[0:1, 0:512]).then_inc(s_mm, 1)
            pe.matmul(p2, w_sb[0:1, F : F + P], w_sb[0:1, 512:F]).then_inc(s_mm, 1)

        @block.scalar
        def _(act: bass.BassEngine):
            act.wait_ge(s_mm, 1)
            act.copy(w_t[:, 0:512], p1)
            act.wait_ge(s_mm, 2)
            act.copy(w_t[:, 512:F], p2).then_inc(s_wt, 1)

        @block.vector
        def _(vec: bass.BassEngine):
            vec.wait_ge(s_wt, 1)
            n_ttr = 0
            for t, (u, ls, n) in enumerate(X_CHUNKS):
                vec.wait_ge(s_x[t % NXSEM], 16 * (t // NXSEM + 1))
                units = [(u, ls, n)] if n <= F else [(u, 0, F), (u + 1, 0, F)]
                for uu, lls, nn in units:
                    if lls > 0:
                        # chained chunk: the previous chunk's TTR (engine
                        # event n_ttr) must have written res[:, uu]
                        vec.wait_ge(s_res, n_ttr)
                        init = res[:, uu : uu + 1]
                    else:
                        init = 0.0
                    vec.tensor_tensor_reduce(
                        out=scr[:, (n_ttr % 2) * F : (n_ttr % 2) * F + nn],
                        in0=xb[:, uu * F + lls : uu * F + lls + nn],
                        in1=w_t[:, lls : lls + nn],
                        scale=1.0,
                        scalar=init,
                        op0=ALU.mult,
                        op1=ALU.add,
                        accum_out=res[:, uu : uu + 1],
                    ).then_inc(s_res, 1)
                    n_ttr += 1

    _cache["nc"] = nc
    return nc


def _make_in_maps(x, lhs, rhs, W, b):
    weff = _fold_weights(lhs, rhs, W)
    wfv = np.concatenate([weff, np.ones(P, np.float32)])
    wfv = np.ascontiguousarray(wfv.reshape(1, F + P))
    xr = np.ascontiguousarray(np.asarray(x, np.float32).reshape(B, F))
    in_maps = []
    for c in range(N_CORES):
        shard = xr[c * B_PER : (c + 1) * B_PER].reshape(NPL, LINE)
        in_maps.append({"xs": shard, "wf": wfv})
    return in_maps


def _assemble(results, b):
    bval = np.float32(np.asarray(b, np.float32).reshape(-1)[0])
    outs = []
    for r in results:
        resm = np.concatenate([r["ys1"], r["ys2"]], axis=1)  # [128, 32]
        # res[p, u] = row (u//8)*1024 + p*8 + u%8
        y = resm.reshape(P, 4, RPP).transpose(1, 0, 2).reshape(B_PER)
        outs.append(y)
    y = np.concatenate(outs) + bval
    return y.reshape(B, 1).astype(np.float32)


def _run(x, lhs, rhs, W, b, **kwargs):
    from concourse.bass_utils import run_bass_kernel_spmd

    nc = _build_program()
    in_maps = _make_in_maps(x, lhs, rhs, W, b)
    br = run_bass_kernel_spmd(nc, in_maps, list(range(N_CORES)), **kwargs)
    return _assemble(br.results, b), br


def kernel(x, lhs, rhs, W, b):
    try:
        y, _ = _run(x, lhs, rhs, W, b)
    except Exception:
        # transient NRT/axon failures have been observed to clear on retry
        y, _ = _run(x, lhs, rhs, W, b)
    return y


# revision 4
# speedup vs baseline: 1.0742x; 1.0051x over previous
"""Trainium2 Bass kernel for nn_CompressorModel (block decompression + linear head).

The reference is linear in x: y[b] = W_eff . x_row[b] + bias, with W_eff[768]
folded on the host from (lhs, rhs, W) in fp64.  Device work is the pure
memory-bound matvec res = X @ W_eff, batch-sharded 8 ways (4096 rows/core).

Per-core program (raw bass, one sync-wait per instruction):
  SP    : HWDGE x-tile DMAs, tapered 2-unit -> 1-unit -> half -> quarter so the
          DVE lag drains before the tail; plus the early output DMA (units
          0..23) once s_res reaches 24.
  Pool  : w-vector DMA [1,896] (w_eff 768 + ones 128, ~10ns of bus), kvwb
          PREPARE for the final output block, and the end-of-stream
          trigger_dma -- the triggered path skips the 565+625+650ns DMA issue
          pipeline on the critical tail.
  PE    : broadcasts w to 128 partitions via ones^T @ w into PSUM (replaces a
          1092ns [128,768] broadcast DMA with a 10ns [1,896] load).
  ACT   : copies PSUM -> w_t[128,768].
  DVE   : one fused tensor_tensor_reduce per chunk: product to scratch,
          accum_out=res[:,u] with scalar-chaining for sub-unit chunks
          (scalar=res[:,u] carries the partial).  861ns/unit vs 1092ns/unit
          arrival, so DVE trails the DMA stream and the tail is just
          900 (sem prop) + one 192-lane TTR + trigger + fire.

A unit u = (block bl=u//8, row-pos c=u%8): 128 partitions = lines
bl*128..bl*128+127 of xs[512, 6144], free = line cols [c*768,(c+1)*768).
res[p, u] = dot of batch row (bl*128+p)*8 + c.  Bias is added on the host.
"""

from contextlib import ExitStack

import numpy as np

B = 32768
N_CORES = 8
B_PER = B // N_CORES          # 4096 rows per core
F = 768                       # 3*16*16 features per row
RPP = 8                       # rows packed per partition line
NPL = B_PER // RPP            # 512 partition lines per core
LINE = RPP * F                # 6144 elems per partition line
P = 128                       # SBUF partitions
UNITS = 32
NXSEM = 8                     # rotating x-DMA completion sems

# x-DMA schedule: (unit, lane_start, lane_count) per DMA, bus order.
# 2-unit tiles are encoded as (unit, 0, 1536).  All chunks keep the DRAM
# contiguous run >= 512B so the cost model charges flat 360GB/s.
X_CHUNKS = []
for j in range(8):                       # units 0..15: 2-unit tiles
    X_CHUNKS.append((2 * j, 0, 2 * F))
for u in range(16, 24):                  # units 16..23: 1-unit tiles
    X_CHUNKS.append((u, 0, F))
for u in range(24, 30):                  # units 24..29: halves
    X_CHUNKS.append((u, 0, F // 2))
    X_CHUNKS.append((u, F // 2, F // 2))
for u in range(30, 32):                  # units 30..31: quarters
    for q in range(4):
        X_CHUNKS.append((u, q * (F // 4), F // 4))
N_XDMA = len(X_CHUNKS)

_cache = {}


def _fold_weights(lhs, rhs, W):
    """W_eff[ch, r*8+p, c*8+q] = sum_{P,Q} lhs[r,P,p]*rhs[c,q,Q]*W[0, ch*1024+(r*16+P)*32+(c*16+Q)]"""
    Wb = np.asarray(W, np.float64).reshape(3, 2, 16, 2, 16)  # [ch, r, P, c, Q]
    weff = np.einsum(
        "rPp,cqQ,nrPcQ->nrpcq",
        np.asarray(lhs, np.float64),
        np.asarray(rhs, np.float64),
        Wb,
    )
    return np.ascontiguousarray(weff.reshape(F)).astype(np.float32)


def _build_program():
    if "nc" in _cache:
        return _cache["nc"]
    from concourse import bass, mybir

    f32 = mybir.dt.float32
    i32 = mybir.dt.int32
    ALU = mybir.AluOpType

    nc = bass.Bass("TRN2", target_bir_lowering=False, debug=False)
    xs = nc.dram_tensor("xs", [NPL, LINE], f32, kind="ExternalInput").ap()
    wf = nc.dram_tensor("wf", [1, F + P], f32, kind="ExternalInput").ap()
    ys1 = nc.dram_tensor("ys1", [P, 24], f32, kind="ExternalOutput").ap()
    ys2 = nc.dram_tensor("ys2", [P, 8], f32, kind="ExternalOutput").ap()

    xb = nc.alloc_sbuf_tensor("xb", [P, UNITS * F], f32).ap()
    w_sb = nc.alloc_sbuf_tensor("w_sb", [1, F + P], f32).ap()
    w_t = nc.alloc_sbuf_tensor("w_t", [P, F], f32).ap()
    res = nc.alloc_sbuf_tensor("res", [P, UNITS], f32).ap()
    scr = nc.alloc_sbuf_tensor("scr", [P, 2 * F], f32).ap()
    idx = nc.alloc_sbuf_tensor("idx", [P, 1], i32).ap()

    p1 = nc.alloc_psum_tensor("p1", [P, 512], f32).ap()
    p2 = nc.alloc_psum_tensor("p2", [P, F - 512], f32).ap()

    # 4D views for kv_writeback: out [batch=1, dhi=128, dho=8, nctx=1],
    # in [dhi=128, dho=8, batch=1, ncn=1].
    ys2_4d = ys2.rearrange("(a p) (f b) -> a p f b", a=1, b=1)
    res_4d = res[:, 24:32].rearrange("p (f a b) -> p f a b", a=1, b=1)

    def xs_src(u, ls, n):
        bl, c = divmod(u, RPP)
        return xs[bl * P : (bl + 1) * P, c * F + ls : c * F + ls + n]

    with (
        nc.Block() as block,
        nc.semaphore("s_x0") as s_x0,
        nc.semaphore("s_x1") as s_x1,
        nc.semaphore("s_x2") as s_x2,
        nc.semaphore("s_x3") as s_x3,
        nc.semaphore("s_x4") as s_x4,
        nc.semaphore("s_x5") as s_x5,
        nc.semaphore("s_x6") as s_x6,
        nc.semaphore("s_x7") as s_x7,
        nc.semaphore("s_w") as s_w,
        nc.semaphore("s_mm") as s_mm,
        nc.semaphore("s_wt") as s_wt,
        nc.semaphore("s_res") as s_res,
        nc.semaphore("s_prep") as s_prep,
        nc.semaphore("s_kv") as s_kv,
        nc.semaphore("s_o1") as s_o1,
    ):
        s_x = [s_x0, s_x1, s_x2, s_x3, s_x4, s_x5, s_x6, s_x7]

        @block.sync
        def _(sp: bass.BassEngine):
            for t, (u, ls, n) in enumerate(X_CHUNKS):
                sp.dma_start(
                    out=xb[:, u * F + ls : u * F + ls + n], in_=xs_src(u, ls, n)
                ).then_inc(s_x[t % NXSEM], 16)
            sp.wait_ge(s_res, 24)
            sp.dma_start(out=ys1, in_=res[:, 0:24]).then_inc(s_o1, 16)
            sp.wait_ge(s_o1, 16)

        @block.gpsimd
        def _(gp: bass.BassEngine):
            gp.dma_start(out=w_sb, in_=wf).then_inc(s_w, 16)
            gp.memset(idx, 0)
            gp.kv_writeback(
                ys2_4d, res_4d, idx, prepare_only=True, sem=s_kv
            ).then_inc(s_prep, 1)
            gp.wait_ge(s_prep, 1)
            gp.wait_ge(s_res, len(X_CHUNKS) + 8)  # all TTRs done (see DVE count)
            gp.trigger_dma(count=1)
            gp.wait_ge(s_kv, 16)

        @block.tensor
        def _(pe: bass.BassEngine):
            pe.wait_ge(s_w, 16)
            pe.matmul(p1, w_sb[0:1, F : F + P], w_sb[0:1, 0:512]).then_inc(s_mm, 1)
            pe.matmul(p2, w_sb[0:1, F : F + P], w_sb[0:1, 512:F]).then_inc(s_mm, 1)

        @block.scalar
        def _(act: bass.BassEngine):
            act.wait_ge(s_mm, 1)
            act.copy(w_t[:, 0:512], p1)
            act.wait_ge(s_mm, 2)
            act.copy(w_t[:, 512:F], p2).then_inc(s_wt, 1)

        @block.vector
        def _(vec: bass.BassEngine):
            vec.wait_ge(s_wt, 1)
            n_ttr = 0
            for t, (u, ls, n) in enumerate(X_CHUNKS):
                vec.wait_ge(s_x[t % NXSEM], 16 * (t // NXSEM + 1))
                units = [(u, ls, n)] if n <= F else [(u, 0, F), (u + 1, 0, F)]
                for uu, lls, nn in units:
                    if lls > 0:
                        # chained chunk: the previous chunk's TTR (engine
                        # event n_ttr) must have written res[:, uu]
                        vec.wait_ge(s_res, n_ttr)
                        init = res[:, uu : uu + 1]
                    else:
                        init = 0.0
                    vec.tensor_tensor_reduce(
                        out=scr[:, (n_ttr % 2) * F : (n_ttr % 2) * F + nn],
                        in0=xb[:, uu * F + lls : uu * F + lls + nn],
                        in1=w_t[:, lls : lls + nn],
                        scale=1.0,
                        scalar=init,
                        op0=ALU.mult,
                        op1=ALU.add,
                        accum_out=res[:, uu : uu + 1],
                    ).then_inc(s_res, 1)
                    n_ttr += 1

    _cache["nc"] = nc
    return nc


def _make_in_maps(x, lhs, rhs, W, b):
    weff = _fold_weights(lhs, rhs, W)
    wfv = np.concatenate([weff, np.ones(P, np.float32)])
    wfv = np.ascontiguousarray(wfv.reshape(1, F + P))
    xr = np.ascontiguousarray(np.asarray(x, np.float32).reshape(B, F))
    in_maps = []
    for c in range(N_CORES):
        shard = xr[c * B_PER : (c + 1) * B_PER].reshape(NPL, LINE)
        in_maps.append({"xs": shard, "wf": wfv})
    return in_maps


def _assemble(results, b):
    bval = np.float32(np.asarray(b, np.float32).reshape(-1)[0])
    outs = []
    for r in results:
        resm = np.concatenate([r["ys1"], r["ys2"]], axis=1)  # [128, 32]
        # res[p, u] = row (u//8)*1024 + p*8 + u%8
        y = resm.reshape(P, 4, RPP).transpose(1, 0, 2).reshape(B_PER)
        outs.append(y)
    y = np.concatenate(outs) + bval
    return y.reshape(B, 1).astype(np.float32)


def _run(x, lhs, rhs, W, b, **kwargs):
    from concourse.bass_utils import run_bass_kernel_spmd

    nc = _build_program()
    in_maps = _make_in_maps(x, lhs, rhs, W, b)
    br = run_bass_kernel_spmd(nc, in_maps, list(range(N_CORES)), **kwargs)
    return _assemble(br.results, b), br


def kernel(x, lhs, rhs, W, b):
    try:
        y, _ = _run(x, lhs, rhs, W, b)
    except Exception:
        # transient NRT/axon failures have been observed to clear on retry
        y, _ = _run(x, lhs, rhs, W, b)
    return y


# revision 6
# speedup vs baseline: 1.1238x; 1.0461x over previous
"""Trainium2 Bass kernel for nn_CompressorModel (block decompression + linear head).

The reference is linear in x: y[b] = W_eff . x_row[b] + bias, with W_eff[768]
folded on the host from (lhs, rhs, W) in fp64.  Device work is the pure
memory-bound matvec res = X @ W_eff, batch-sharded 8 ways (4096 rows/core).

Per-core program (raw bass, one sync-wait per instruction):
  SP    : HWDGE x-tile DMAs, tapered 2-unit -> 1-unit -> half -> quarter so the
          DVE lag drains before the tail; plus the early output DMA (units
          0..23) once s_res reaches 24.
  Pool  : w-vector DMA [1,896] (w_eff 768 + ones 128, ~10ns of bus), kvwb
          PREPARE for the final output block, and the end-of-stream
          trigger_dma -- the triggered path skips the 565+625+650ns DMA issue
          pipeline on the critical tail.
  PE    : broadcasts w to 128 partitions via ones^T @ w into PSUM (replaces a
          1092ns [128,768] broadcast DMA with a 10ns [1,896] load).
  ACT   : copies PSUM -> w_t[128,768].
  DVE   : one fused tensor_tensor_reduce per chunk: product to scratch,
          accum_out=res[:,u] with scalar-chaining for sub-unit chunks
          (scalar=res[:,u] carries the partial).  861ns/unit vs 1092ns/unit
          arrival, so DVE trails the DMA stream and the tail is just
          900 (sem prop) + one 192-lane TTR + trigger + fire.

A unit u = (block bl=u//8, row-pos c=u%8): 128 partitions = lines
bl*128..bl*128+127 of xs[512, 6144], free = line cols [c*768,(c+1)*768).
res[p, u] = dot of batch row (bl*128+p)*8 + c.  Bias is added on the host.
"""

from contextlib import ExitStack

import numpy as np

B = 32768
N_CORES = 8
B_PER = B // N_CORES          # 4096 rows per core
F = 768                       # 3*16*16 features per row
RPP = 8                       # rows packed per partition line
NPL = B_PER // RPP            # 512 partition lines per core
LINE = RPP * F                # 6144 elems per partition line
P = 128                       # SBUF partitions
UNITS = 32
NXSEM = 8                     # rotating x-DMA completion sems

# x-DMA schedule: (unit, lane_start, lane_count) per DMA, bus order.
# 2-unit tiles are encoded as (unit, 0, 1536).  All chunks keep the DRAM
# contiguous run >= 512B so the cost model charges flat 360GB/s.  Sub-unit
# chunks of neighbouring units are interleaved pairwise so each chained
# TTR's sem round-trip hides behind the partner unit's TTR.
X_CHUNKS = []
for j in range(8):                       # units 0..15: 2-unit tiles
    X_CHUNKS.append((2 * j, 0, 2 * F))
for u in range(16, 24):                  # units 16..23: 1-unit tiles
    X_CHUNKS.append((u, 0, F))
for u in range(24, 30, 2):               # units 24..29: interleaved halves
    for h in range(2):
        X_CHUNKS.append((u, h * (F // 2), F // 2))
        X_CHUNKS.append((u + 1, h * (F // 2), F // 2))
for q in range(4):                       # units 30..31: interleaved quarters
    X_CHUNKS.append((30, q * (F // 4), F // 4))
    X_CHUNKS.append((31, q * (F // 4), F // 4))
N_XDMA = len(X_CHUNKS)

_cache = {}


def _fold_weights(lhs, rhs, W):
    """W_eff[ch, r*8+p, c*8+q] = sum_{P,Q} lhs[r,P,p]*rhs[c,q,Q]*W[0, ch*1024+(r*16+P)*32+(c*16+Q)]"""
    Wb = np.asarray(W, np.float64).reshape(3, 2, 16, 2, 16)  # [ch, r, P, c, Q]
    weff = np.einsum(
        "rPp,cqQ,nrPcQ->nrpcq",
        np.asarray(lhs, np.float64),
        np.asarray(rhs, np.float64),
        Wb,
    )
    return np.ascontiguousarray(weff.reshape(F)).astype(np.float32)


def _build_program():
    if "nc" in _cache:
        return _cache["nc"]
    from concourse import bass, mybir

    f32 = mybir.dt.float32
    i32 = mybir.dt.int32
    ALU = mybir.AluOpType

    nc = bass.Bass("TRN2", target_bir_lowering=False, debug=False)
    xs = nc.dram_tensor("xs", [NPL, LINE], f32, kind="ExternalInput").ap()
    wf = nc.dram_tensor("wf", [1, F + P], f32, kind="ExternalInput").ap()
    ys1 = nc.dram_tensor("ys1", [P, 24], f32, kind="ExternalOutput").ap()
    ys2 = nc.dram_tensor("ys2", [P, 8], f32, kind="ExternalOutput").ap()

    xb = nc.alloc_sbuf_tensor("xb", [P, UNITS * F], f32).ap()
    w_sb = nc.alloc_sbuf_tensor("w_sb", [1, F + P], f32).ap()
    w_t = nc.alloc_sbuf_tensor("w_t", [P, F], f32).ap()
    res = nc.alloc_sbuf_tensor("res", [P, UNITS], f32).ap()
    scr = nc.alloc_sbuf_tensor("scr", [P, 2 * F], f32).ap()
    idx = nc.alloc_sbuf_tensor("idx", [P, 1], i32).ap()

    p1 = nc.alloc_psum_tensor("p1", [P, 512], f32).ap()
    p2 = nc.alloc_psum_tensor("p2", [P, F - 512], f32).ap()

    # 4D views for kv_writeback: out [batch=1, dhi=128, dho=8, nctx=1],
    # in [dhi=128, dho=8, batch=1, ncn=1].
    ys2_4d = ys2.rearrange("(a p) (f b) -> a p f b", a=1, b=1)
    res_4d = res[:, 24:32].rearrange("p (f a b) -> p f a b", a=1, b=1)

    def xs_src(u, ls, n):
        bl, c = divmod(u, RPP)
        return xs[bl * P : (bl + 1) * P, c * F + ls : c * F + ls + n]

    with (
        nc.Block() as block,
        nc.semaphore("s_x0") as s_x0,
        nc.semaphore("s_x1") as s_x1,
        nc.semaphore("s_x2") as s_x2,
        nc.semaphore("s_x3") as s_x3,
        nc.semaphore("s_x4") as s_x4,
        nc.semaphore("s_x5") as s_x5,
        nc.semaphore("s_x6") as s_x6,
        nc.semaphore("s_x7") as s_x7,
        nc.semaphore("s_w") as s_w,
        nc.semaphore("s_mm") as s_mm,
        nc.semaphore("s_wt") as s_wt,
        nc.semaphore("s_res") as s_res,
        nc.semaphore("s_prep") as s_prep,
        nc.semaphore("s_kv") as s_kv,
        nc.semaphore("s_o1") as s_o1,
    ):
        s_x = [s_x0, s_x1, s_x2, s_x3, s_x4, s_x5, s_x6, s_x7]

        @block.sync
        def _(sp: bass.BassEngine):
            for t, (u, ls, n) in enumerate(X_CHUNKS):
                sp.dma_start(
                    out=xb[:, u * F + ls : u * F + ls + n], in_=xs_src(u, ls, n)
                ).then_inc(s_x[t % NXSEM], 16)
            sp.wait_ge(s_res, 24)
            sp.dma_start(out=ys1, in_=res[:, 0:24]).then_inc(s_o1, 16)
            sp.wait_ge(s_o1, 16)

        @block.gpsimd
        def _(gp: bass.BassEngine):
            gp.dma_start(out=w_sb, in_=wf).then_inc(s_w, 16)
            gp.memset(idx, 0)
            gp.kv_writeback(
                ys2_4d, res_4d, idx, prepare_only=True, sem=s_kv
            ).then_inc(s_prep, 1)
            gp.wait_ge(s_prep, 1)
            gp.wait_ge(s_res, len(X_CHUNKS) + 8)  # all TTRs done (see DVE count)
            gp.trigger_dma(count=1)
            gp.wait_ge(s_kv, 16)

        @block.tensor
        def _(pe: bass.BassEngine):
            pe.wait_ge(s_w, 16)
            pe.matmul(p1, w_sb[0:1, F : F + P], w_sb[0:1, 0:512]).then_inc(s_mm, 1)
            pe.matmul(p2, w_sb[0:1, F : F + P], w_sb[0:1, 512:F]).then_inc(s_mm, 1)

        @block.scalar
        def _(act: bass.BassEngine):
            act.wait_ge(s_mm, 1)
            act.copy(w_t[:, 0:512], p1)
            act.wait_ge(s_mm, 2)
            act.copy(w_t[:, 512:F], p2).then_inc(s_wt, 1)

        @block.vector
        def _(vec: bass.BassEngine):
            vec.wait_ge(s_wt, 1)
            n_ttr = 0
            last_ttr = {}  # unit -> 1-based index of its latest TTR
            for t, (u, ls, n) in enumerate(X_CHUNKS):
                vec.wait_ge(s_x[t % NXSEM], 16 * (t // NXSEM + 1))
                units = [(u, ls, n)] if n <= F else [(u, 0, F), (u + 1, 0, F)]
                for uu, lls, nn in units:
                    if lls > 0:
                        # chained chunk: wait for this unit's previous TTR to
                        # have written res[:, uu] (interleaving makes this a
                        # non-stalling lagged wait)
                        vec.wait_ge(s_res, last_ttr[uu])
                        init = res[:, uu : uu + 1]
                    else:
                        init = 0.0
                    vec.tensor_tensor_reduce(
                        out=scr[:, (n_ttr % 2) * F : (n_ttr % 2) * F + nn],
                        in0=xb[:, uu * F + lls : uu * F + lls + nn],
                        in1=w_t[:, lls : lls + nn],
                        scale=1.0,
                        scalar=init,
                        op0=ALU.mult,
                        op1=ALU.add,
                        accum_out=res[:, uu : uu + 1],
                    ).then_inc(s_res, 1)
                    n_ttr += 1
                    last_ttr[uu] = n_ttr

    _cache["nc"] = nc
    return nc


def _make_in_maps(x, lhs, rhs, W, b):
    weff = _fold_weights(lhs, rhs, W)
    wfv = np.concatenate([weff, np.ones(P, np.float32)])
    wfv = np.ascontiguousarray(wfv.reshape(1, F + P))
    xr = np.ascontiguousarray(np.asarray(x, np.float32).reshape(B, F))
    in_maps = []
    for c in range(N_CORES):
        shard = xr[c * B_PER : (c + 1) * B_PER].reshape(NPL, LINE)
        in_maps.append({"xs": shard, "wf": wfv})
    return in_maps


def _assemble(results, b):
    bval = np.float32(np.asarray(b, np.float32).reshape(-1)[0])
    outs = []
    for r in results:
        resm = np.concatenate([r["ys1"], r["ys2"]], axis=1)  # [128, 32]
        # res[p, u] = row (u//8)*1024 + p*8 + u%8
        y = resm.reshape(P, 4, RPP).transpose(1, 0, 2).reshape(B_PER)
        outs.append(y)
    y = np.concatenate(outs) + bval
    return y.reshape(B, 1).astype(np.float32)


def _run(x, lhs, rhs, W, b, **kwargs):
    from concourse.bass_utils import run_bass_kernel_spmd

    nc = _build_program()
    in_maps = _make_in_maps(x, lhs, rhs, W, b)
    br = run_bass_kernel_spmd(nc, in_maps, list(range(N_CORES)), **kwargs)
    return _assemble(br.results, b), br


def kernel(x, lhs, rhs, W, b):
    try:
        y, _ = _run(x, lhs, rhs, W, b)
    except Exception:
        # transient NRT/axon failures have been observed to clear on retry
        y, _ = _run(x, lhs, rhs, W, b)
    return y
